# revision 22
# baseline (speedup 1.0000x reference)
"""Trainium2 Bass kernel: causal multi-head self-attention (B=2, T=2048, C=1024, H=16).

Sharding: 8 cores = 2 batch groups x 4 head groups. Core c handles batch c//4,
heads [4*(c%4), 4*(c%4)+4). Each core computes QKV for its 4 heads, head-wise
causal attention, and a partial output projection (contraction over its 256
channels). Host sums the 4 partials per batch and adds bp.

All transposes are done host-side (numpy); the device kernel only ever sees
matmul-friendly layouts:
  xt  [8,128,2048]  x[b].T packed into 8 k-tiles (c = k*128 + p)
  wq/wk/wv [8,128,256]  W[mine,:].T k-packed   (lhsT / rhs for projections)
  wp  [2,128,1024]      Wp[:,mine].T k-packed
Device layouts:
  Q.T/K.T [128, 2(m-tile), 2048]  (head h: m-tile h//2, partitions 64*(h%2)+d)
  V       [128, 16(t-tile), 256]  (head h: cols 64h..64h+64)
Attention per 512-query i-tile, 128-key j-tile (causal tiles only):
  S.T[j,i] matmuls (row-packed K=64 head pairs) -> PSUM [128, 4*512]
  P.T = exp(S.T/8 + kp_bias_j)  (no-max softmax; ScalarE, per-partition bias)
  diagonal tiles masked by a precomputed 0/1 triangle multiply
  Y.T += V_h.T-packed matmuls (col-packed pairs); rowsums via M=1 ones-matmuls
  Y.T scaled by 1/rowsum (partition_broadcast + multiply) -> proj matmuls.
"""

import numpy as np

N_CORES = 8
B, T, C = 2, 2048, 1024
H, D = 16, 64
HPC = 4          # heads per core
MC = HPC * D     # 256 per-core channels
KT = C // 128    # 8 contraction k-tiles
NT = T // 512    # 4 query i-tiles (512 wide)
NJ = T // 128    # 16 key j-tiles (128 wide)

MODE = "f32r"    # "f32r" | "bf16"  (matmul operand dtype)
REPEAT = 1       # unrolled body repetitions (timing builds only)
NEG = -1.0e30

_BUILT = {}      # (mode, repeat) -> nc
_RUNNER = {}     # (mode, repeat) -> callable(in_maps) -> results list


def _dt(mode):
    from concourse import mybir
    return mybir.dt.float32r if mode == "f32r" else mybir.dt.bfloat16


def _np_dt(mode):
    if mode == "f32r":
        return np.float32
    import ml_dtypes
    return ml_dtypes.bfloat16


def build_nc(mode=MODE, repeat=REPEAT, dbg=False):
    import concourse.bass as bass
    import concourse.tile as tile
    from concourse import bacc, mybir

    f32 = mybir.dt.float32
    dm = _dt(mode)            # matmul-operand storage dtype

    def mm_ap(ap):
        # matmul-feeding tiles are already float32r / bf16 typed
        return ap

    nc = bacc.Bacc("TRN2", target_bir_lowering=False, debug=False,
                   enable_asserts=False, num_devices=N_CORES)

    xt_d = nc.dram_tensor("xt", [KT, 128, T], dm, kind="ExternalInput").ap()
    wq_d = nc.dram_tensor("wq", [KT, 128, MC], dm, kind="ExternalInput").ap()
    wk_d = nc.dram_tensor("wk", [KT, 128, MC], dm, kind="ExternalInput").ap()
    wv_d = nc.dram_tensor("wv", [KT, 128, MC], dm, kind="ExternalInput").ap()
    wp_d = nc.dram_tensor("wp", [2, 128, C], dm, kind="ExternalInput").ap()
    bq_d = nc.dram_tensor("bq", [128, 2], f32, kind="ExternalInput").ap()
    bk_d = nc.dram_tensor("bk", [128, 2], f32, kind="ExternalInput").ap()
    bv_d = nc.dram_tensor("bv", [1, MC], f32, kind="ExternalInput").ap()
    kpb_d = nc.dram_tensor("kpb", [128, NJ], f32, kind="ExternalInput").ap()
    zm_d = nc.dram_tensor("zm", [128, 897], dm, kind="ExternalInput").ap()
    out_d = nc.dram_tensor("out", [T, C], f32, kind="ExternalOutput").ap()

    dbg_d = None
    if dbg:
        dbg_d = dict(
            dq=nc.dram_tensor("dq", [128, 2, T], dm, kind="ExternalOutput").ap(),
            dk=nc.dram_tensor("dk", [128, 2, T], dm, kind="ExternalOutput").ap(),
            dv=nc.dram_tensor("dv", [128, NJ, MC], dm, kind="ExternalOutput").ap(),
            dpt=nc.dram_tensor("dpt", [4, 128, 4 * 512], dm, kind="ExternalOutput").ap(),
            drr=nc.dram_tensor("drr", [NT, 128, 512], f32, kind="ExternalOutput").ap(),
            dri=nc.dram_tensor("dri", [NT, 128, 512], f32, kind="ExternalOutput").ap(),
            dysc=nc.dram_tensor("dysc", [NT, 2, 128, 512], f32, kind="ExternalOutput").ap(),
            dyt=nc.dram_tensor("dyt", [NT, 2, 128, 512], dm, kind="ExternalOutput").ap(),
        )

    with tile.TileContext(nc) as tc:
        _emit(nc, tc, mode, repeat, mm_ap, dm, f32,
              xt_d, wq_d, wk_d, wv_d, wp_d, bq_d, bk_d, bv_d, kpb_d, zm_d, out_d,
              dbg_d)

    nc.compile()
    return nc


def _emit(nc, tc, mode, repeat, mm_ap, dm, f32,
          xt_d, wq_d, wk_d, wv_d, wp_d, bq_d, bk_d, bv_d, kpb_d, zm_d, out_d,
          dbg_d=None):
    from contextlib import ExitStack
    import concourse.tile as tile
    from concourse import mybir

    Exp = mybir.ActivationFunctionType.Exp

    with ExitStack() as ctx:
        consts = ctx.enter_context(tc.tile_pool(name="consts", bufs=1))
        # constants / weights
        wq_s = consts.tile([128, KT, MC], dm, name="wq_s")
        wk_s = consts.tile([128, KT, MC], dm, name="wk_s")
        wv_s = consts.tile([128, KT, MC], dm, name="wv_s")
        wp_s = consts.tile([128, 2, C], dm, name="wp_s")
        bq_s = consts.tile([128, 2], f32, name="bq_s")
        bk_s = consts.tile([128, 2], f32, name="bk_s")
        bv_s = consts.tile([1, MC], f32, name="bv_s")
        bvb_s = consts.tile([128, MC], f32, name="bvb_s")
        kpb_s = consts.tile([128, NJ], f32, name="kpb_s")
        zm_s = consts.tile([128, 897], dm, name="zm_s")

        nc.sync.dma_start(wq_s[:], wq_d.rearrange("k p m -> p k m"))
        nc.sync.dma_start(wk_s[:], wk_d.rearrange("k p m -> p k m"))
        nc.sync.dma_start(wv_s[:], wv_d.rearrange("k p m -> p k m"))
        nc.sync.dma_start(wp_s[:], wp_d.rearrange("k p m -> p k m"))
        nc.sync.dma_start(bq_s[:], bq_d)
        nc.sync.dma_start(bk_s[:], bk_d)
        nc.sync.dma_start(bv_s[:], bv_d)
        nc.sync.dma_start(kpb_s[:], kpb_d)
        nc.sync.dma_start(zm_s[:], zm_d)
        nc.gpsimd.partition_broadcast(bvb_s[:], bv_s[0:1, :])

        # persistent activations
        colpack = (mode == "bf16")   # fp32r matmuls reject col tile_positions
        acts = ctx.enter_context(tc.tile_pool(name="acts", bufs=1))
        qt_s = acts.tile([128, 2, T], dm, name="qt_s")
        kt_s = acts.tile([128, 2, T], dm, name="kt_s")
        if colpack:
            v_s = acts.tile([128, NJ, MC], dm, name="v_s")
        else:
            # per-head 65-wide blocks: cols 0:64 = V_h, col 64 = ones
            # (PV matmul M=65 -> row 64 of Y psum accumulates the rowsum)
            v_s = acts.tile([128, NJ, HPC, D + 1], dm, name="v_s")
            nc.scalar.activation(
                v_s[:, :, :, D],
                zm_s[:, 0:NJ * HPC].rearrange("p (a b) -> p a b", b=HPC),
                mybir.ActivationFunctionType.Copy, bias=1.0, scale=0.0)

        rep = 0
        if repeat > 1:
            loop_cm = tc.For_i(0, repeat, 1,
                               hint_engines=(mybir.EngineType.PE,
                                             mybir.EngineType.DVE,
                                             mybir.EngineType.Activation,
                                             mybir.EngineType.SP))
        else:
            import contextlib
            loop_cm = contextlib.nullcontext()
        with loop_cm:
            # ---------------- phase 1: QKV projections ----------------
            with tc.tile_pool(name=f"xs{rep}", bufs=2) as xpool, \
                 tc.tile_pool(name=f"qkvp{rep}", bufs=4, space="PSUM") as qkv_ps:
                for t in range(NT):          # 512-wide t slices
                    xt_t = xpool.tile([128, KT, 512], dm, tag="xt_t")
                    nc.sync.dma_start(
                        xt_t[:], xt_d[:, :, t * 512:(t + 1) * 512]
                        .rearrange("k p t -> p k t"))
                    for m in range(2):
                        qp = qkv_ps.tile([128, 512], f32, tag="qkv")
                        for k in range(KT):
                            nc.tensor.matmul(
                                qp[:], mm_ap(wq_s[:, k, m * 128:(m + 1) * 128]),
                                mm_ap(xt_t[:, k, :]),
                                start=(k == 0), stop=(k == KT - 1))
                        nc.vector.tensor_scalar_add(
                            qt_s[:, m, t * 512:(t + 1) * 512], qp[:],
                            bq_s[:, m:m + 1])
                        kp = qkv_ps.tile([128, 512], f32, tag="qkv")
                        for k in range(KT):
                            nc.tensor.matmul(
                                kp[:], mm_ap(wk_s[:, k, m * 128:(m + 1) * 128]),
                                mm_ap(xt_t[:, k, :]),
                                start=(k == 0), stop=(k == KT - 1))
                        nc.vector.tensor_scalar_add(
                            kt_s[:, m, t * 512:(t + 1) * 512], kp[:],
                            bk_s[:, m:m + 1])
                    for tt in range(4):      # 128-wide rows of V
                        jt = t * 4 + tt
                        vp = qkv_ps.tile([128, 512], f32, tag="qkv")
                        for k in range(KT):
                            nc.tensor.matmul(
                                vp[:, 0:MC],
                                mm_ap(xt_t[:, k, tt * 128:(tt + 1) * 128]),
                                mm_ap(wv_s[:, k, :]),
                                start=(k == 0), stop=(k == KT - 1))
                        if colpack:
                            nc.vector.tensor_add(v_s[:, jt, :], vp[:, 0:MC],
                                                 bvb_s[:])
                        else:
                            nc.vector.tensor_add(
                                v_s[:, jt, :, 0:D],
                                vp[:, 0:MC].rearrange("p (h e) -> p h e", e=D),
                                bvb_s[:].rearrange("p (h e) -> p h e", e=D))

            # ---------------- phase 2+3: attention + projection ----------------
            from contextlib import ExitStack as _ES
            with _ES() as phctx:
                s_ps = phctx.enter_context(
                    tc.tile_pool(name=f"sps{rep}", bufs=1, space="PSUM"))
                if colpack:
                    y_ps = phctx.enter_context(
                        tc.tile_pool(name=f"yps{rep}", bufs=2, space="PSUM"))
                    r_ps = phctx.enter_context(
                        tc.tile_pool(name=f"rps{rep}", bufs=1, space="PSUM"))
                    p_ps = phctx.enter_context(
                        tc.tile_pool(name=f"pps{rep}", bufs=1, space="PSUM"))
                else:
                    y_ps = phctx.enter_context(
                        tc.tile_pool(name=f"yps{rep}", bufs=4, space="PSUM"))
                    p_ps = y_ps
                ppool = phctx.enter_context(
                    tc.tile_pool(name=f"ptile{rep}", bufs=3))
                ypool = phctx.enter_context(
                    tc.tile_pool(name=f"ytile{rep}", bufs=4))
                rpool = phctx.enter_context(
                    tc.tile_pool(name=f"rtile{rep}", bufs=4))
                opool = phctx.enter_context(
                    tc.tile_pool(name=f"otile{rep}", bufs=3))
                for a in range(NT):          # 512-wide query tiles
                    njt = 4 * a + 4
                    if colpack:
                        ya = y_ps.tile([128, 512], f32, tag="y")
                        yb = y_ps.tile([128, 512], f32, tag="y")
                        rr = r_ps.tile([128, 512], f32, tag="r")
                        yh = None
                    else:
                        yh = [y_ps.tile([D + 1, 512], f32, tag="y",
                                        name=f"yh{rep}_{a}_{hh}")
                              for hh in range(HPC)]
                    for jt in range(njt):
                        sp = s_ps.tile([128, 4 * 512], f32, tag="s")
                        for h in range(HPC):
                            lo = 64 * (h % 2)
                            nc.tensor.matmul(
                                sp[:, h * 512:(h + 1) * 512],
                                mm_ap(kt_s[lo:lo + 64, h // 2,
                                           jt * 128:(jt + 1) * 128]),
                                mm_ap(qt_s[lo:lo + 64, h // 2,
                                           a * 512:(a + 1) * 512]),
                                start=True, stop=True, tile_position=(lo, 0))
                        pt = ppool.tile([128, 4 * 512], dm, tag="p")
                        nc.scalar.activation(pt[:], sp[:], Exp,
                                             bias=kpb_s[:, jt:jt + 1],
                                             scale=0.125)
                        r = jt - 4 * a
                        if r >= 0:           # diagonal-band tile: causal mask
                            zsl = zm_s[:, 384 - 128 * r:896 - 128 * r]
                            for h in range(HPC):
                                nc.vector.tensor_mul(
                                    pt[:, h * 512:(h + 1) * 512],
                                    pt[:, h * 512:(h + 1) * 512], zsl)
                        if dbg_d is not None and a == 0 and rep == 0:
                            nc.sync.dma_start(dbg_d["dpt"][jt], pt[:])
                        st = (jt == 0)
                        sto = (jt == njt - 1)
                        if colpack:
                            for h in range(HPC):
                                ypsum = ya if h < 2 else yb
                                lo = 64 * (h % 2)
                                nc.tensor.matmul(
                                    ypsum[lo:lo + 64, :],
                                    mm_ap(v_s[:, jt, h * 64:(h + 1) * 64]),
                                    mm_ap(pt[:, h * 512:(h + 1) * 512]),
                                    start=st, stop=sto, tile_position=(0, lo))
                            for h in range(HPC):
                                nc.tensor.matmul(
                                    rr[32 * h:32 * h + 1, :],
                                    mm_ap(zm_s[:, 896:897]),
                                    mm_ap(pt[:, h * 512:(h + 1) * 512]),
                                    start=st, stop=sto,
                                    tile_position=(0, 32 * h))
                        else:
                            for h in range(HPC):
                                nc.tensor.matmul(
                                    yh[h][:],
                                    mm_ap(v_s[:, jt, h, :]),
                                    mm_ap(pt[:, h * 512:(h + 1) * 512]),
                                    start=st, stop=sto)
                    # 1/rowsum: collect the 4 head rowsum rows at partitions
                    # 0/32/64/96, quadrant-broadcast (stream_shuffle), recip.
                    if colpack:
                        rsrc = rr
                    else:
                        rsrc = rpool.tile([128, 512], f32, tag="rg")
                        for h in range(HPC):
                            nc.vector.tensor_copy(rsrc[32 * h:32 * h + 1, :],
                                                  yh[h][D:D + 1, :])
                    s4 = rpool.tile([128, 512], f32, tag="s4")
                    nc.vector.stream_shuffle(s4[:], rsrc[:], [0] * 32)
                    ysc = rpool.tile([128, 512], f32, tag="ysc")
                    nc.vector.reciprocal(ysc[:], s4[:])
                    yta = ypool.tile([128, 512], dm, tag="yt")
                    ytb = ypool.tile([128, 512], dm, tag="yt")
                    if dbg_d is not None and rep == 0:
                        nc.sync.dma_start(dbg_d["dri"][a], ysc[:])
                    for h in range(HPC):
                        ytile = yta if h < 2 else ytb
                        lo = 64 * (h % 2)
                        for half in range(2):
                            o = lo + 32 * half
                            if colpack:
                                ypsum = ya if h < 2 else yb
                                src = ypsum[o:o + 32, :]
                            else:
                                src = yh[h][32 * half:32 * half + 32, :]
                            nc.vector.tensor_mul(
                                ytile[o:o + 32, :], src,
                                ysc[32 * h:32 * h + 32, :])
                    if dbg_d is not None and rep == 0:
                        nc.sync.dma_start(dbg_d["dyt"][a, 0], yta[:])
                        nc.sync.dma_start(dbg_d["dyt"][a, 1], ytb[:])
                    # output projection for this i-tile
                    for s in range(4):
                        ot = opool.tile([128, C], f32, tag="ot")
                        for o in range(2):
                            if colpack:
                                pp = p_ps.tile([128, 512], f32, tag="pp")
                            else:
                                pp = p_ps.tile([128, 512], f32, tag="y")
                            nc.tensor.matmul(
                                pp[:], mm_ap(yta[:, s * 128:(s + 1) * 128]),
                                mm_ap(wp_s[:, 0, o * 512:(o + 1) * 512]),
                                start=True, stop=False)
                            nc.tensor.matmul(
                                pp[:], mm_ap(ytb[:, s * 128:(s + 1) * 128]),
                                mm_ap(wp_s[:, 1, o * 512:(o + 1) * 512]),
                                start=False, stop=True)
                            nc.vector.tensor_copy(
                                ot[:, o * 512:(o + 1) * 512], pp[:])
                        nc.sync.dma_start(
                            out_d[a * 512 + s * 128:a * 512 + (s + 1) * 128, :],
                            ot[:])
            if dbg_d is not None and rep == 0:
                nc.sync.dma_start(dbg_d["dq"][:], qt_s[:])
                nc.sync.dma_start(dbg_d["dk"][:], kt_s[:])
                nc.sync.dma_start(dbg_d["dv"][:], v_s[:])


def make_in_maps(inputs, mode=MODE):
    npdt = _np_dt(mode)
    x = np.asarray(inputs["x"], np.float32)
    kpm = np.asarray(inputs["key_padding_mask"])
    Wq = np.asarray(inputs["Wq"], np.float32)
    Wk = np.asarray(inputs["Wk"], np.float32)
    Wv = np.asarray(inputs["Wv"], np.float32)
    Wp = np.asarray(inputs["Wp"], np.float32)
    bq = np.asarray(inputs["bq"], np.float32)
    bk = np.asarray(inputs["bk"], np.float32)
    bv = np.asarray(inputs["bv"], np.float32)

    zm = np.concatenate([np.zeros((128, 384)), np.triu(np.ones((128, 128))),
                         np.ones((128, 384 + 1))], axis=1).astype(npdt)
    in_maps = []
    for c in range(N_CORES):
        b, g = c // 4, c % 4
        hs = slice(g * MC, (g + 1) * MC)
        xt = np.ascontiguousarray(x[b].T).reshape(KT, 128, T).astype(npdt)
        wq = np.ascontiguousarray(Wq[hs, :].T).reshape(KT, 128, MC).astype(npdt)
        wk = np.ascontiguousarray(Wk[hs, :].T).reshape(KT, 128, MC).astype(npdt)
        wv = np.ascontiguousarray(Wv[hs, :].T).reshape(KT, 128, MC).astype(npdt)
        wp = np.ascontiguousarray(Wp[:, hs].T).reshape(2, 128, C).astype(npdt)
        bqc = np.ascontiguousarray(bq[hs].reshape(2, 128).T).astype(np.float32)
        bkc = np.ascontiguousarray(bk[hs].reshape(2, 128).T).astype(np.float32)
        bvc = bv[hs].reshape(1, MC).astype(np.float32)
        kpb = np.ascontiguousarray(
            np.where(kpm[b], NEG, 0.0).astype(np.float32).reshape(NJ, 128).T)
        in_maps.append(dict(xt=xt, wq=wq, wk=wk, wv=wv, wp=wp,
                            bq=bqc, bk=bkc, bv=bvc, kpb=kpb, zm=zm))
    return in_maps


def get_runner(mode=MODE, repeat=REPEAT, dbg=False):
    """Build (once) and return callable(in_maps) -> list of per-core out dicts."""
    key = (mode, repeat, dbg)
    if key in _RUNNER:
        return _RUNNER[key]
    if key not in _BUILT:
        _BUILT[key] = build_nc(mode, repeat, dbg)
    nc = _BUILT[key]

    import jax
    from concourse import mybir
    from concourse.bass2jax import (_bass_exec_p, install_neuronx_cc_hook,
                                    partition_id_tensor)
    from jax.sharding import Mesh, PartitionSpec
    try:
        from jax.experimental.shard_map import shard_map
    except ImportError:
        from jax.shard_map import shard_map

    install_neuronx_cc_hook()
    partition_name = (nc.partition_id_tensor.name
                      if nc.partition_id_tensor else None)
    in_names, out_names, out_avals = [], [], []
    for alloc in nc.m.functions[0].allocations:
        if not isinstance(alloc, mybir.MemoryLocationSet):
            continue
        name = alloc.memorylocations[0].name
        if alloc.kind == "ExternalInput":
            if name != partition_name:
                in_names.append(name)
        elif alloc.kind == "ExternalOutput":
            out_names.append(name)
            out_avals.append(jax.core.ShapedArray(
                tuple(alloc.tensor_shape), mybir.dt.np(alloc.dtype)))
    n_params = len(in_names)
    n_outs = len(out_names)
    all_names = in_names + out_names
    if partition_name is not None:
        all_names = all_names + [partition_name]
    donate = tuple(range(n_params, n_params + n_outs))

    def _body(*args):
        operands = list(args)
        if partition_name is not None:
            operands.append(partition_id_tensor())
        return tuple(_bass_exec_p.bind(
            *operands, out_avals=tuple(out_avals), in_names=tuple(all_names),
            out_names=tuple(out_names), lowering_input_output_aliases=(),
            sim_require_finite=True, sim_require_nnan=True, nc=nc))



    devices = jax.devices()[:N_CORES]
    mesh = Mesh(np.asarray(devices), ("core",))
    sharded = jax.jit(
        shard_map(_body, mesh=mesh,
                  in_specs=(PartitionSpec("core"),) * (n_params + n_outs),
                  out_specs=(PartitionSpec("core"),) * n_outs,
                  check_rep=False),
        donate_argnums=donate, keep_unused=True)

    def run(in_maps):
        concat_in = [np.concatenate([np.asarray(m[i]) for m in in_maps], axis=0)
                     for i in in_names]
        concat_zeros = [np.zeros((N_CORES * av.shape[0], *av.shape[1:]), av.dtype)
                        for av in out_avals]
        outs = sharded(*concat_in, *concat_zeros)
        return [
            {name: np.asarray(outs[i]).reshape(N_CORES, *out_avals[i].shape)[c]
             for i, name in enumerate(out_names)}
            for c in range(N_CORES)
        ]

    sharded_z = jax.jit(
        shard_map(_body, mesh=mesh,
                  in_specs=(PartitionSpec("core"),) * (n_params + n_outs),
                  out_specs=(PartitionSpec("core"),) * n_outs,
                  check_rep=False))

    def bench(in_maps, iters=10):
        """Device-resident timing: returns list of per-call wall seconds."""
        import time as _time
        concat_in = [np.concatenate([np.asarray(m[i]) for m in in_maps], axis=0)
                     for i in in_names]
        concat_zeros = [np.zeros((N_CORES * av.shape[0], *av.shape[1:]), av.dtype)
                        for av in out_avals]
        sh = jax.sharding.NamedSharding(mesh, PartitionSpec("core"))
        dev_in = [jax.device_put(a, sh) for a in concat_in + concat_zeros]
        outs = sharded_z(*dev_in)
        jax.block_until_ready(outs)
        times = []
        for _ in range(iters):
            t0 = _time.perf_counter()
            outs = sharded_z(*dev_in)
            jax.block_until_ready(outs)
            times.append(_time.perf_counter() - t0)
        return times

    run.bench = bench
    _RUNNER[key] = run
    return run


def kernel(**inputs):
    run = get_runner(MODE, 1)
    in_maps = make_in_maps(inputs, MODE)
    results = run(in_maps)
    bp = np.asarray(inputs["bp"], np.float32)
    out = np.zeros((B, T, C), np.float32)
    for c in range(N_CORES):
        out[c // 4] += results[c]["out"]
    out += bp[None, None, :]
    return out
